# revision 38
# baseline (speedup 1.0000x reference)
"""Trainium2 Bass kernel for GQA attention layer (B=1, S=2048, H=4096,
32 Q heads / 8 KV heads, head_dim 128, RoPE with arbitrary tables).

Sharding: tensor-parallel over heads across 8 NeuronCores — core c gets
Q heads 4c..4c+3 and KV head c (Wq/Wk/Wv column shards, Wo row shard).
Each core computes its partial o_proj output [2048, 4096]; the host sums
the 8 partials (equivalent of the all-reduce).

Schedule: a single software pipeline over q-ranges. Section qr emits the
attention inner loop for q-range qr (scores -> exp -> PV, flash-style,
kt-pair PSUM tiles so one ACT exp covers 1024 columns) interleaved with
the QKV projection + RoPE matmuls of q-range qr+1, so the ACT engine's
exp throughput hides entirely under the PE-bound projection stream. The
last section (qr=3) has no projection work left, so o_proj matmuls of
completed q-ranges fill the PE gaps instead; the remainder drains after,
rotating accumulators across all 8 PSUM banks.

Other specifics:
  - A(0) (the unoverlapped prologue projections) is emitted CHUNK-major:
    per hidden-chunk c, all 6 jobs (k, v, q0..3) issue one matmul each,
    against a host-side combined weight tensor wqkv[128, KC, 6, 128]
    DMA'd in c-groups interleaved with hst chunks across both rings.
    PE demand (~250GB/s) then stays below the feed rate, so the first
    matmul fires as soon as the first 2-chunk group lands (~11us) and
    runs gap-free, instead of stalling until the whole 5MB k-job input
    is resident. k/q0/q1 run 1-2 chunks ahead in the step schedule so
    their rope evicts overlap the tail of the c-loop.
  - RoPE rotate-half is a pswap permutation matmul back into the job's
    own dead PSUM accumulator (an SBUF-SBUF DMA or gpsimd op in the
    middle of the rope chain stalls the in-order DVE queue, which
    cascades into PE sem waits), with the whole rope evaluated in bf16
    on the DVE (2x mode).
  - softmax denominator: probs pairs folded on DVE into a bf16
    accumulator (2x mode), partition-reduced with one bf16 ones-matmul
    (1 cycle/row vs 4 for fp32), then fast-reciprocal + gpsimd
    partition_broadcast + DVE multiply normalize the PV accumulator.
  - diagonal k-tiles narrow their scores/exp/PV to the unmasked column
    range plus one triangular 128-col mask multiply.
  - output partials are stored bf16 (halves DMA-out and SBUF staging);
    DMAs alternate between the sync and scalar rings during the drain;
    the final o_proj tile is split into 128-col pieces so the last
    evict+DMA chain after the last matmul is ~4x shorter.  The host
    sums the 8 partials in fp32.
"""

import sys
from contextlib import ExitStack

sys.path.insert(0, "/opt/trn_rl_repo")

import numpy as np
import ml_dtypes

import concourse.bass as bass
import concourse.bacc as bacc
import concourse.mybir as mybir
import concourse.tile as tile
from concourse import bass_isa
from concourse.bass_utils import run_bass_kernel_spmd
from concourse.masks import make_identity

BF16 = mybir.dt.bfloat16
F32 = mybir.dt.float32
F32R = mybir.dt.float32r

N_CORES = 8
S = 2048
HID = 4096
D = 128
NQ = 4  # q heads per core
KC = HID // 128  # 32 hidden-dim chunks
NQR = S // 512  # 4 q ranges of 512
NST = S // 128  # 16 s-tiles of 128
NHO = HID // 512  # 8 output column tiles of 512
SCALE = 1.0 / float(np.sqrt(D))

_CACHE: dict = {}


def _build_nc():
    nc = bacc.Bacc(None, target_bir_lowering=False, debug=False)

    hst_d = nc.dram_tensor("hst", [NQR, 128, KC, 512], BF16, kind="ExternalInput")
    wqkv_d = nc.dram_tensor("wqkv", [128, KC, 6, D], BF16, kind="ExternalInput")
    wo_d = nc.dram_tensor("wo", [128, NQ, HID], BF16, kind="ExternalInput")
    cos_d = nc.dram_tensor("cos2", [128, S], BF16, kind="ExternalInput")
    sin_d = nc.dram_tensor("sin2", [128, S], BF16, kind="ExternalInput")
    out_d = nc.dram_tensor("out", [S, HID], BF16, kind="ExternalOutput")

    with tile.TileContext(nc) as tc, ExitStack() as stack:
        # ---- persistent SBUF pools ----
        const = stack.enter_context(tc.tile_pool(name="const", bufs=1))
        act = stack.enter_context(tc.tile_pool(name="act", bufs=1))
        qt_sb = [
            act.tile([128, S], BF16, tag=f"qt{h}", name=f"qt{h}") for h in range(NQ)
        ]
        kt_sb = act.tile([128, S], BF16, tag="kt")
        vt_sb = act.tile([128, S], BF16, tag="vt")
        v_sb = act.tile([128, NST, 128], BF16, tag="v")  # [s,d] chunks per k-tile
        attn_sb = [
            act.tile([128, S], BF16, tag=f"attn{h}", name=f"attn{h}")
            for h in range(NQ)
        ]
        wqkv_p = stack.enter_context(tc.tile_pool(name="wqkv", bufs=1))
        hstp = stack.enter_context(tc.tile_pool(name="hstp", bufs=2))
        rope = stack.enter_context(tc.tile_pool(name="rope", bufs=2))
        probs_p = stack.enter_context(tc.tile_pool(name="probs", bufs=3))
        den_p = stack.enter_context(tc.tile_pool(name="den", bufs=2))
        bcast_p = stack.enter_context(tc.tile_pool(name="bcast", bufs=2))
        ostage = stack.enter_context(tc.tile_pool(name="ostage", bufs=6))

        # ---- PSUM pools: 2 + 4 + 2 = 8 banks ----
        psA = stack.enter_context(tc.tile_pool(name="psA", bufs=2, space="PSUM"))
        psS = stack.enter_context(tc.tile_pool(name="psS", bufs=2, space="PSUM"))
        psO = stack.enter_context(tc.tile_pool(name="psO", bufs=2, space="PSUM"))

        # ================= prologue DMAs =================
        # Consumption order is chunk-major, so both rings stream c-groups
        # in lockstep:
        #   sync ring:   wqkv c-groups (then hst2/hst3/wo later)
        #   scalar ring: hst0 c-groups, cos/sin[qr0], hst1, cos/sin rest
        # Leading 2-chunk groups let the first matmuls start as soon as
        # the first ~650KB lands.
        hst_tiles: list = [None] * NQR
        hst_t0 = hstp.tile([128, KC, 512], BF16, tag="hst", name="hst0")
        hst_tiles[0] = hst_t0
        wqkv_sb = wqkv_p.tile([128, KC, 6, D], BF16)
        cos_sb = const.tile([128, S], BF16)
        sin_sb = const.tile([128, S], BF16)

        # 2-chunk groups: the k-stream in A(0) runs up to 10 chunks
        # ahead of the step index, so supply must track need at fine
        # grain (4-chunk groups put c~16-24 ~2us late).  The leading
        # 1-chunk groups + step-0's c0-first emission order let the
        # first matmul start ~1.5us earlier.
        c_groups = [(0, 1), (1, 2)] + [(c, c + 2) for c in range(2, KC, 2)]
        for lo, hi in c_groups:
            nc.sync.dma_start(wqkv_sb[:, lo:hi], wqkv_d[:, lo:hi])
            nc.scalar.dma_start(hst_t0[:, lo:hi, :], hst_d[0, :, lo:hi, :])
        nc.scalar.dma_start(cos_sb[:, 0:512], cos_d[:, 0:512])
        nc.scalar.dma_start(sin_sb[:, 0:512], sin_d[:, 0:512])
        nc.scalar.dma_start(cos_sb[:, 512:S], cos_d[:, 512:S])
        nc.scalar.dma_start(sin_sb[:, 512:S], sin_d[:, 512:S])
        # hst1 rides sync AFTER the wqkv groups: its transfers would
        # otherwise share the aggregate DMA bandwidth with A(0)'s
        # supply-critical late wqkv chunks (both rings pull from the
        # same 16 engines); A(1) only needs it from ~56us.
        hst_t1 = hstp.tile([128, KC, 512], BF16, tag="hst", name="hst1")
        hst_tiles[1] = hst_t1
        for r in range(4):
            nc.sync.dma_start(
                hst_t1[:, r * 8 : (r + 1) * 8, :],
                hst_d[1, :, r * 8 : (r + 1) * 8, :],
            )

        def hst_chunks(dst, qr):
            for r in range(4):
                nc.sync.dma_start(
                    dst[:, r * 8 : (r + 1) * 8, :],
                    hst_d[qr, :, r * 8 : (r + 1) * 8, :],
                )

        # wo is allocated late, into hst2's hstp slot (dead once A(2) is
        # emitted) — SBUF is too tight to hold both for the whole kernel.
        wo_ref: dict = {}

        # weight chunk views: j=0 -> wk, j=1 -> wv, j=2+h -> wq head h
        def w_of(kind, h, c):
            if kind == "k":
                return wqkv_sb[:, c, 0, :]
            if kind == "v":
                return wqkv_sb[:, c, 1, :]
            return wqkv_sb[:, c, 2 + h, :]

        # ---- gpsimd-built constants ----
        identity = const.tile([128, 128], BF16)
        make_identity(nc, identity[:])
        ones_b = const.tile([128, 1], BF16)
        nc.gpsimd.memset(ones_b[:], 1.0)
        # triangular mask for the diagonal 128x128 subtile: rows are k,
        # cols are q; keep q >= k.
        tri = const.tile([128, 128], BF16)
        nc.gpsimd.memset(tri[:], 1.0)
        nc.gpsimd.affine_select(
            out=tri[:],
            in_=tri[:],
            pattern=[[1, 128]],
            compare_op=mybir.AluOpType.is_ge,
            fill=0.0,
            base=0,
            channel_multiplier=-1,
        )
        # pswap: permutation matrix swapping partition halves, so the RoPE
        # rotate-half is one PE matmul (no DVE-queue stall)
        pswap = const.tile([128, 128], BF16)
        ptmp = const.tile([128, 128], BF16)
        nc.gpsimd.memset(pswap[:], 1.0)
        nc.gpsimd.memset(ptmp[:], 1.0)
        nc.gpsimd.affine_select(
            out=pswap[:],
            in_=pswap[:],
            pattern=[[1, 128]],
            compare_op=mybir.AluOpType.is_equal,
            fill=0.0,
            base=64,
            channel_multiplier=-1,
        )
        nc.gpsimd.affine_select(
            out=ptmp[:],
            in_=ptmp[:],
            pattern=[[1, 128]],
            compare_op=mybir.AluOpType.is_equal,
            fill=0.0,
            base=-64,
            channel_multiplier=-1,
        )
        nc.gpsimd.tensor_add(pswap[:], pswap[:], ptmp[:])

        # ================= A-phase building blocks =================
        def rope_copy(ps):
            """First half of a rope evict: snapshot the PSUM accumulator
            to SBUF.  Split from rope_rest so the pswap matmul can be
            emitted a step later and never waits on the DVE."""
            raw = rope.tile([128, 512], BF16, tag="raw")
            nc.vector.tensor_copy(raw[:], ps[:])
            return raw

        def rope_rest(raw, ps, dst_tile, qr):
            """dst[0:64]  = x0*cos - x1*sin
            dst[64:128] = x1*cos + x0*sin   (x0=ps[0:64], x1=ps[64:128]).
            The rotate-half is a pswap permutation matmul back into the
            job's own (now dead) PSUM accumulator — no DMA, and no wait
            embedded in the in-order DVE queue."""
            sl = slice(qr * 512, (qr + 1) * 512)
            nc.tensor.matmul(
                ps[:], pswap[:], raw[:], start=True, stop=True,
                skip_group_check=True,
            )
            m1 = rope.tile([128, 512], BF16, tag="m1")
            nc.vector.tensor_mul(m1[:], raw[:], cos_sb[:, sl])
            m2 = rope.tile([128, 512], BF16, tag="m2")
            nc.vector.tensor_mul(m2[:], ps[:], sin_sb[:, sl])
            # sin table's top half is pre-negated host-side, so the
            # combine is one full-width add instead of sub + add
            nc.vector.tensor_add(dst_tile[:, sl], m1[:], m2[:])

        def rope_evict(ps, dst_tile, qr):
            rope_rest(rope_copy(ps), ps, dst_tile, qr)

        def emit_a0():
            """Chunk-major A(0): per step, ~one matmul per job against
            the combined wqkv chunk.  Job finish-steps are staggered two
            steps apart (k@21, q0@23, q1@25, v@27, q2@29, q3@31, via
            doubled chunks on early steps) so each ~2.6us DVE rope-evict
            chain hides under the next job's remaining matmul stream —
            at full-chunk-major the six serial chains stall the in-order
            PE queue ~7us at the A(0)->B(0) boundary.  Returns thunks
            (q3 rope tail, last transposes) for B(0)'s fill slots."""
            ps_k = psA.tile([128, 512], F32, tag="a", name="a0_k")
            ps_v = psA.tile([128, 512], F32, tag="a", name="a0_v")
            ps_q01 = psS.tile([128, 1024], F32, tag="s", name="a0_q01")
            ps_q23 = psS.tile([128, 1024], F32, tag="s", name="a0_q23")
            raws: dict = {}
            order = ["k", "q0", "q1", "v", "q2", "q3"]
            streams = {
                "k": (ps_k, slice(0, 512), "k", 0),
                "v": (ps_v, slice(0, 512), "v", 0),
                "q0": (ps_q01, slice(0, 512), "q", 0),
                "q1": (ps_q01, slice(512, 1024), "q", 1),
                "q2": (ps_q23, slice(0, 512), "q", 2),
                "q3": (ps_q23, slice(512, 1024), "q", 3),
            }
            fin = {s: 21 + 2 * j for j, s in enumerate(order)}
            dbl = {s: KC - 1 - fin[s] for s in order}  # doubled early steps

            def mm(sname, c):
                ps, cols, kind, h = streams[sname]
                nc.tensor.matmul(
                    ps[:, cols],
                    w_of(kind, h, c),
                    hst_t0[:, c, :],
                    start=(c == 0),
                    stop=(c == KC - 1),
                    skip_group_check=True,
                )

            def evict_begin(sname):
                ps, cols, kind, _ = streams[sname]
                if kind == "v":
                    nc.vector.tensor_copy(vt_sb[:, 0:512], ps[:, cols])
                else:
                    raws[sname] = rope_copy(ps[:, cols])

            def evict_end(sname):
                ps, cols, kind, _ = streams[sname]
                if kind == "v":
                    return
                dst = kt_sb if kind == "k" else qt_sb[streams[sname][3]]
                rope_rest(raws[sname], ps[:, cols], dst, 0)

            def transp(kt):
                pst = psA.tile([128, 128], BF16, tag="a", name=f"vt{kt}")
                nc.tensor.transpose(
                    pst[:], vt_sb[:, kt * 128 : (kt + 1) * 128], identity[:]
                )
                nc.vector.tensor_copy(v_sb[:, kt, :], pst[:])

            for step in range(KC):
                if step == 0:
                    # c0 across all streams first: the first matmuls only
                    # need the leading 1-chunk DMA group
                    for sname in order:
                        mm(sname, 0)
                    for sname in order:
                        if dbl[sname] > 0:
                            mm(sname, 1)
                else:
                    for sname in order:
                        d, f = dbl[sname], fin[sname]
                        if step < d:
                            mm(sname, 2 * step)
                            mm(sname, 2 * step + 1)
                        elif step <= f:
                            mm(sname, step + d)
                for sname in order:
                    if step == fin[sname]:
                        evict_begin(sname)
                    elif step == fin[sname] + 1:
                        evict_end(sname)
                if step >= 29:
                    transp(step - 29)
            # q3's evict_begin fired at step 31; its rope tail and the
            # last transpose go first into section 0's fill slots

            def rest_q3():
                evict_end("q3")

            def transp3():
                transp(3)

            return [("pe", rest_q3), ("pe", transp3)]

        def a_units(qr):
            """Yield thunks for A(qr), qr>=1: 6 projection jobs in
            c-chunks of 8 matmuls, evictions, and the v transposes for
            this qr.  Each job's evict is delayed until after the NEXT
            job's first chunk so the DVE's PSUM read (which frees the
            accumulator bank) has slack."""
            hst_t = hst_tiles[qr]
            jobs = [("k", 0), ("v", 0)] + [("q", h) for h in range(NQ)]
            state: dict = {}
            pending = []

            for kind, h in jobs:

                def alloc(kind=kind, h=h):
                    state[(kind, h)] = psA.tile(
                        [128, 512], F32, tag="a", name=f"a{qr}_{kind}{h}"
                    )

                for cg in range(4):

                    def chunk(kind=kind, h=h, cg=cg, alloc=alloc):
                        if cg == 0:
                            alloc()
                        ps = state[(kind, h)]
                        for c in range(cg * 8, cg * 8 + 8):
                            nc.tensor.matmul(
                                ps[:],
                                w_of(kind, h, c),
                                hst_t[:, c, :],
                                start=(c == 0),
                                stop=(c == KC - 1),
                                skip_group_check=True,
                            )

                    yield ("pe", chunk)
                    if cg == 0 and pending:
                        for u in pending:
                            yield u
                        pending = []

                def evict(kind=kind, h=h):
                    ps = state.pop((kind, h))
                    if kind == "q":
                        rope_evict(ps, qt_sb[h], qr)
                    elif kind == "k":
                        rope_evict(ps, kt_sb, qr)
                    else:
                        sl = slice(qr * 512, (qr + 1) * 512)
                        nc.vector.tensor_copy(vt_sb[:, sl], ps[:])

                if kind == "v":
                    # v's evict + transposes stay immediate: the transpose
                    # scratch tiles must chain through both psA slots
                    # before the next job's accumulator is allocated
                    yield ("dve", evict)
                    for kt in range(qr * 4, qr * 4 + 4):

                        def transp(kt=kt):
                            pst = psA.tile(
                                [128, 128], BF16, tag="a", name=f"vt{kt}"
                            )
                            nc.tensor.transpose(
                                pst[:],
                                vt_sb[:, kt * 128 : (kt + 1) * 128],
                                identity[:],
                            )
                            nc.vector.tensor_copy(v_sb[:, kt, :], pst[:])

                        yield ("pe", transp)
                else:
                    pending.append(("dve", evict))

            for u in pending:
                yield u

        # ================= o_proj (phase C) machinery =================
        # the very last (st, ho) tile is split into 128-col pieces so the
        # final evict+DMA chain after the final matmul is short
        def c_units():
            for qrC in range(NQR):
                for st in range(qrC * 4, qrC * 4 + 4):
                    for ho in range(NHO):
                        last = st == NST - 1 and ho == NHO - 1
                        pieces = (
                            [(0, 512)]
                            if not last
                            else [(i * 128, (i + 1) * 128) for i in range(4)]
                        )
                        for c0, c1 in pieces:
                            yield ("alloc", qrC, st, ho, c0, c1)
                            for h in range(NQ):
                                yield ("mm", qrC, st, ho, c0, c1, h)
                            yield ("evict", qrC, st, ho, c0, c1)

        c_state = {"gen": c_units(), "pending": None, "tile": None, "nalloc": 0,
                   "nevict": 0}

        def c_alloc_tile(st, ho, w, wide):
            """Rotate accumulators over psA only (fill mode) or all three
            PSUM pools (drain mode)."""
            i = c_state["nalloc"]
            c_state["nalloc"] += 1
            if not wide:
                return psA.tile([128, w], F32, tag="a", name=f"c{st}_{ho}")
            # drain mode: rotate over all 8 banks (2 per pool tag, the
            # "s" tiles are 2 banks each) so bank reuse is ~3 units out
            which = i % 6
            if which in (0, 3):
                return psA.tile([128, w], F32, tag="a", name=f"c{st}_{ho}")
            if which in (1, 4):
                return psO.tile([128, w], F32, tag="o", name=f"c{st}_{ho}")
            return psS.tile([128, 1024], F32, tag="s", name=f"c{st}_{ho}")

        def emit_c(n_mms, qr_done, wide=False):
            emitted = 0
            while emitted < n_mms:
                unit = c_state["pending"] or next(c_state["gen"], None)
                c_state["pending"] = None
                if unit is None:
                    return False
                if unit[1] > qr_done:
                    c_state["pending"] = unit
                    return False
                if unit[0] == "alloc":
                    _, _, st, ho, c0, c1 = unit
                    c_state["tile"] = c_alloc_tile(st, ho, c1 - c0, wide)
                elif unit[0] == "mm":
                    _, _, st, ho, c0, c1, h = unit
                    nc.tensor.matmul(
                        c_state["tile"][:, 0 : c1 - c0],
                        attn_sb[h][:, st * 128 : (st + 1) * 128],
                        wo_ref["wo"][:, h, ho * 512 + c0 : ho * 512 + c1],
                        start=(h == 0),
                        stop=(h == NQ - 1),
                        skip_group_check=True,
                    )
                    emitted += 1
                else:
                    _, _, st, ho, c0, c1 = unit
                    i = c_state["nevict"]
                    c_state["nevict"] += 1
                    w = c1 - c0
                    stg = ostage.tile([128, w], BF16, tag="stg")
                    if not wide or i % 3 != 1:
                        # fill mode keeps ACT free — it paces the B(3) exps
                        nc.vector.tensor_copy(stg[:], c_state["tile"][:, 0:w])
                    else:
                        nc.scalar.copy(stg[:], c_state["tile"][:, 0:w])
                    # drain outputs rotate over sync/scalar/gpsimd (all
                    # inputs done by then; gpsimd's queue is empty, so its
                    # triggers fire the moment the stage copy lands);
                    # fills ride sync (scalar is still running B(3) exps).
                    # The last few tiles avoid gpsimd — its end-of-kernel
                    # queue DRAIN detects completion ~2us slower than the
                    # rings and would gate the postamble.
                    if not wide:
                        eng = nc.sync
                    elif i > 120:
                        eng = (nc.sync, nc.scalar)[i % 2]
                    else:
                        eng = (nc.sync, nc.scalar, nc.gpsimd)[i % 3]
                    eng.dma_start(
                        out_d[
                            st * 128 : (st + 1) * 128,
                            ho * 512 + c0 : ho * 512 + c1,
                        ],
                        stg[:],
                    )
            return True

        # ================= B-phase building blocks =================
        # (note: moving the denominator partition-reduce to gpsimd
        # partition_all_reduce was tried twice and regresses ~20us: its
        # ~3.5us latency lands in the in-order DVE queue — the DVE runs
        # AHEAD of real time on the folds, so even a head of emission
        # distance doesn't hide it, and every PE wait on a DVE count
        # slips.  The PE ones-matmul is in-stream and effectively free.)
        def b_units(qr):
            """Yield thunks for the attention inner loop of q-range qr.
            Each pair-visit thunk emits: scores prefetch for the next
            pair + exp, then (after the scheduler's filler) PV + folds."""
            n_kt = 4 * (qr + 1)
            n_pair = n_kt // 2
            qsl = slice(qr * 512, (qr + 1) * 512)

            def c0_of(kt):
                p_idx = kt - 4 * qr
                return 128 * p_idx if p_idx > 0 else 0

            for h in range(NQ):
                st8: dict = {}

                def mm_scores_pair(j, h=h, st8=st8):
                    ps_s = psS.tile(
                        [128, 1024], F32, tag="s", name=f"s{qr}_{h}_{j}"
                    )
                    for idx in range(2):
                        kt = 2 * j + idx
                        c0 = c0_of(kt)
                        nc.tensor.matmul(
                            ps_s[:, idx * 512 + c0 : (idx + 1) * 512],
                            kt_sb[:, kt * 128 : (kt + 1) * 128],
                            qt_sb[h][:, qr * 512 + c0 : (qr + 1) * 512],
                            start=True,
                            stop=True,
                            skip_group_check=True,
                        )
                    st8[j] = ps_s

                def act_exp(j, h=h, st8=st8):
                    ps_s = st8[j]
                    pt = probs_p.tile(
                        [128, 1024], BF16, tag="pt", name=f"pt{qr}_{h}_{j}"
                    )
                    if 2 * j + 1 < 4 * qr:
                        nc.scalar.activation(
                            pt[:],
                            ps_s[:],
                            mybir.ActivationFunctionType.Exp,
                            scale=SCALE,
                        )
                    else:
                        for idx in range(2):
                            kt = 2 * j + idx
                            c0 = c0_of(kt)
                            nc.scalar.activation(
                                pt[:, idx * 512 + c0 : (idx + 1) * 512],
                                ps_s[:, idx * 512 + c0 : (idx + 1) * 512],
                                mybir.ActivationFunctionType.Exp,
                                scale=SCALE,
                            )
                            nc.vector.tensor_mul(
                                pt[:, idx * 512 + c0 : idx * 512 + c0 + 128],
                                pt[:, idx * 512 + c0 : idx * 512 + c0 + 128],
                                tri[:],
                            )
                    st8[("pt", j)] = pt

                def pv_folds(j, h=h, st8=st8):
                    pt = st8.pop(("pt", j))
                    ps_o = st8["o"]
                    den_acc = st8["d"]
                    for idx in range(2):
                        kt = 2 * j + idx
                        c0 = c0_of(kt)
                        nc.tensor.matmul(
                            ps_o[:, c0:512],
                            v_sb[:, kt, :],
                            pt[:, idx * 512 + c0 : (idx + 1) * 512],
                            start=(kt == 0),
                            stop=(kt == n_kt - 1),
                            skip_group_check=True,
                        )
                    pts = probs_p.tile(
                        [128, 512], BF16, tag="pts", name=f"pts{qr}_{h}_{j}"
                    )
                    c0a, c0b = c0_of(2 * j), c0_of(2 * j + 1)
                    if c0b > c0a:
                        nc.vector.tensor_copy(pts[:, c0a:c0b], pt[:, c0a:c0b])
                    nc.vector.tensor_add(
                        pts[:, c0b:512],
                        pt[:, c0b:512],
                        pt[:, 512 + c0b : 1024],
                    )
                    if j == 0:
                        nc.vector.tensor_copy(den_acc[:], pts[:])
                    else:
                        nc.vector.tensor_add(
                            den_acc[:, c0a:512],
                            den_acc[:, c0a:512],
                            pts[:, c0a:512],
                        )

                def head_begin(h=h, st8=st8, msp=mm_scores_pair, ae=act_exp):
                    st8["o"] = psO.tile(
                        [128, 512], F32, tag="o", name=f"o{qr}_{h}"
                    )
                    # bf16 accumulator: the DVE folds hit 2x mode.  Costs
                    # ~0.5% on the denominator (sqrt(n_pair) bf16 rounds).
                    st8["d"] = den_p.tile(
                        [128, 512], BF16, tag="da", name=f"da{qr}_{h}"
                    )
                    msp(0)
                    ae(0)

                yield ("pe", head_begin)
                # filler right after the first scores pair: keeps the PE
                # fed while ACT runs exp(0) at head/section starts
                yield ("fill", None)
                for j in range(n_pair):

                    def prefetch(j=j, n_pair=n_pair, msp=mm_scores_pair,
                                 ae=act_exp):
                        if j + 1 < n_pair:
                            msp(j + 1)
                            ae(j + 1)

                    yield ("pe", prefetch)
                    yield ("fill", None)

                    def fin(j=j, pf=pv_folds):
                        pf(j)

                    yield ("pe", fin)

                def head_end(h=h, st8=st8, n_pair=n_pair):
                    ps_s = st8[n_pair - 1]
                    den_acc = st8["d"]
                    # partition-reduce at bf16 matmul speed (1 cycle/row
                    # vs 4 for fp32)
                    nc.tensor.matmul(
                        ps_s[0:1, 0:512],
                        ones_b[:],
                        den_acc[:],
                        start=True,
                        stop=True,
                        skip_group_check=True,
                    )
                    recip = den_p.tile([1, 512], F32, tag="recip", name=f"rc{qr}_{h}")
                    nc.vector.reciprocal_approx_fast(out=recip[:], in_=ps_s[0:1, 0:512])
                    bc = bcast_p.tile([128, 512], F32, tag="bc")
                    nc.gpsimd.partition_broadcast(bc[:], recip[:])
                    nc.vector.tensor_mul(
                        attn_sb[h][:, qsl], st8["o"][:], bc[:]
                    )

                yield ("pe", head_end)

        # ================= the pipeline =================
        # Solid A(0) first (nothing to overlap with yet); its v
        # transposes are deferred into section 0's fill slots.
        a0_transp = emit_a0()

        for qr in range(NQR):
            if qr + 2 < NQR:
                nxt = hstp.tile(
                    [128, KC, 512], BF16, tag="hst", name=f"hst{qr+2}"
                )
                hst_tiles[qr + 2] = nxt
                hst_chunks(nxt, qr + 2)

            au = list(a_units(qr + 1)) if qr + 1 < NQR else []
            if qr == 0:
                # interleave A(1)'s k chunks (dependency-free PE cover)
                # with A(0)'s rope tail + last transpose, which wait on
                # the still-draining DVE evict chains
                au = [
                    au[0], a0_transp[0], au[1], a0_transp[1], au[2],
                ] + au[3:]
            bu = list(b_units(qr))
            n_fill = sum(1 for k, _ in bu if k == "fill")
            ai = 0
            fills_done = 0
            for k, thunk in bu:
                if k == "fill":
                    fills_done += 1
                    if au:
                        # distribute A units evenly over the fill slots
                        target = (len(au) * fills_done) // n_fill
                        while ai < target:
                            au[ai][1]()
                            ai += 1
                    else:
                        emit_c(2, qr - 1, wide=False)
                else:
                    thunk()
            while ai < len(au):
                au[ai][1]()
                ai += 1

            if qr == 1:
                # A(2) fully emitted — hst2's slot is reclaimable for wo
                wo_sb = hstp.tile([128, NQ, HID], BF16, tag="hst", name="wo_sb")
                wo_ref["wo"] = wo_sb
                for h in range(NQ):
                    nc.sync.dma_start(wo_sb[:, h, :], wo_d[:, h, :])

        # ---- drain the remaining o_proj work ----
        while emit_c(4, NQR - 1, wide=True):
            pass

    nc.compile()
    return nc


def _get_nc():
    if "nc" not in _CACHE:
        _CACHE["nc"] = _build_nc()
    return _CACHE["nc"]


def _bf16(x):
    return np.ascontiguousarray(x.astype(ml_dtypes.bfloat16))


def _prep_in_maps(hidden_states, sin_table, cos_table, Wq, Wk, Wv, Wo):
    hs0 = np.asarray(hidden_states, np.float32).reshape(S, HID)
    # hst[qr, p, c, s] = hs0[qr*512 + s, c*128 + p]
    hst = _bf16(hs0.reshape(NQR, 512, KC, 128).transpose(0, 3, 2, 1))
    cosT = np.asarray(cos_table, np.float32).T  # [64, S]
    sinT = np.asarray(sin_table, np.float32).T
    cos2 = _bf16(np.concatenate([cosT, cosT], 0))  # [128, S]
    # top half negated: rope combine is then a single full-width add
    sin2 = _bf16(np.concatenate([-sinT, sinT], 0))
    Wq = np.asarray(Wq, np.float32)
    Wk = np.asarray(Wk, np.float32)
    Wv = np.asarray(Wv, np.float32)
    Wo = np.asarray(Wo, np.float32)

    in_maps = []
    for c in range(N_CORES):
        wq_c = Wq[:, c * 512 : (c + 1) * 512]  # 4 q heads
        wk_c = Wk[:, c * 128 : (c + 1) * 128]  # 1 kv head
        wv_c = Wv[:, c * 128 : (c + 1) * 128]
        wo_c = Wo[c * 512 : (c + 1) * 512, :]  # matching rows
        # combined chunk-major weights: wqkv[p, c, j, d] with j = 0:k,
        # 1:v, 2+h:q head h; element = W_j[c*128 + p, (head offset+) d]
        wqkv = np.empty((128, KC, 6, D), np.float32)
        wqkv[:, :, 0, :] = wk_c.reshape(KC, 128, D).swapaxes(0, 1)
        wqkv[:, :, 1, :] = wv_c.reshape(KC, 128, D).swapaxes(0, 1)
        wq_l = wq_c.reshape(KC, 128, NQ, D).transpose(2, 1, 0, 3)  # [h,p,c,d]
        for h in range(NQ):
            wqkv[:, :, 2 + h, :] = wq_l[h]
        in_maps.append(
            {
                "hst": hst,
                "wqkv": _bf16(wqkv),
                "wo": _bf16(wo_c.reshape(NQ, 128, HID).swapaxes(0, 1)),
                "cos2": cos2,
                "sin2": sin2,
            }
        )
    return in_maps


def run(trace=False, **inputs):
    nc = _get_nc()
    in_maps = _prep_in_maps(**inputs)
    res = run_bass_kernel_spmd(
        nc, in_maps, core_ids=list(range(N_CORES)), trace=trace
    )
    partials = np.stack(
        [np.asarray(res.results[c]["out"], np.float32) for c in range(N_CORES)]
    )
    out = partials.sum(axis=0, dtype=np.float32).reshape(1, S, HID)
    return out, res


def kernel(**inputs):
    out, _ = run(trace=False, **inputs)
    return out


# revision 40
# speedup vs baseline: 1.0124x; 1.0124x over previous
"""Trainium2 Bass kernel for GQA attention layer (B=1, S=2048, H=4096,
32 Q heads / 8 KV heads, head_dim 128, RoPE with arbitrary tables).

Sharding: tensor-parallel over heads across 8 NeuronCores — core c gets
Q heads 4c..4c+3 and KV head c (Wq/Wk/Wv column shards, Wo row shard).
Each core computes its partial o_proj output [2048, 4096]; the host sums
the 8 partials (equivalent of the all-reduce).

Schedule: a single software pipeline over q-ranges. Section qr emits the
attention inner loop for q-range qr (scores -> exp -> PV, flash-style,
kt-pair PSUM tiles so one ACT exp covers 1024 columns) interleaved with
the QKV projection + RoPE matmuls of q-range qr+1, so the ACT engine's
exp throughput hides entirely under the PE-bound projection stream. The
last section (qr=3) has no projection work left, so o_proj matmuls of
completed q-ranges fill the PE gaps instead; the remainder drains after,
rotating accumulators across all 8 PSUM banks.

Other specifics:
  - A(0) (the unoverlapped prologue projections) is emitted CHUNK-major:
    per hidden-chunk c, all 6 jobs (k, v, q0..3) issue one matmul each,
    against a host-side combined weight tensor wqkv[128, KC, 6, 128]
    DMA'd in c-groups interleaved with hst chunks across both rings.
    PE demand (~250GB/s) then stays below the feed rate, so the first
    matmul fires as soon as the first 2-chunk group lands (~11us) and
    runs gap-free, instead of stalling until the whole 5MB k-job input
    is resident. k/q0/q1 run 1-2 chunks ahead in the step schedule so
    their rope evicts overlap the tail of the c-loop.
  - RoPE rotate-half is a pswap permutation matmul back into the job's
    own dead PSUM accumulator (an SBUF-SBUF DMA or gpsimd op in the
    middle of the rope chain stalls the in-order DVE queue, which
    cascades into PE sem waits), with the whole rope evaluated in bf16
    on the DVE (2x mode).
  - softmax denominator: probs pairs folded on DVE into a bf16
    accumulator (2x mode), partition-reduced with one bf16 ones-matmul
    (1 cycle/row vs 4 for fp32), then fast-reciprocal + gpsimd
    partition_broadcast + DVE multiply normalize the PV accumulator.
  - diagonal k-tiles narrow their scores/exp/PV to the unmasked column
    range plus one triangular 128-col mask multiply.
  - output partials are stored bf16 (halves DMA-out and SBUF staging);
    DMAs alternate between the sync and scalar rings during the drain;
    the final o_proj tile is split into 128-col pieces so the last
    evict+DMA chain after the last matmul is ~4x shorter.  The host
    sums the 8 partials in fp32.
"""

import sys
from contextlib import ExitStack

sys.path.insert(0, "/opt/trn_rl_repo")

import numpy as np
import ml_dtypes

import concourse.bass as bass
import concourse.bacc as bacc
import concourse.mybir as mybir
import concourse.tile as tile
from concourse import bass_isa
from concourse.bass_utils import run_bass_kernel_spmd
from concourse.masks import make_identity

BF16 = mybir.dt.bfloat16
F32 = mybir.dt.float32
F32R = mybir.dt.float32r

N_CORES = 8
S = 2048
HID = 4096
D = 128
NQ = 4  # q heads per core
KC = HID // 128  # 32 hidden-dim chunks
NQR = S // 512  # 4 q ranges of 512
NST = S // 128  # 16 s-tiles of 128
NHO = HID // 512  # 8 output column tiles of 512
SCALE = 1.0 / float(np.sqrt(D))

_CACHE: dict = {}


def _build_nc():
    nc = bacc.Bacc(None, target_bir_lowering=False, debug=False)

    hst_d = nc.dram_tensor("hst", [NQR, 128, KC, 512], BF16, kind="ExternalInput")
    wqkv_d = nc.dram_tensor("wqkv", [128, KC, 6, D], BF16, kind="ExternalInput")
    wo_d = nc.dram_tensor("wo", [128, NQ, HID], BF16, kind="ExternalInput")
    cos_d = nc.dram_tensor("cos2", [128, S], BF16, kind="ExternalInput")
    sin_d = nc.dram_tensor("sin2", [128, S], BF16, kind="ExternalInput")
    out_d = nc.dram_tensor("out", [S, HID], BF16, kind="ExternalOutput")

    with tile.TileContext(nc) as tc, ExitStack() as stack:
        # ---- persistent SBUF pools ----
        const = stack.enter_context(tc.tile_pool(name="const", bufs=1))
        act = stack.enter_context(tc.tile_pool(name="act", bufs=1))
        qt_sb = [
            act.tile([128, S], BF16, tag=f"qt{h}", name=f"qt{h}") for h in range(NQ)
        ]
        kt_sb = act.tile([128, S], BF16, tag="kt")
        vt_sb = act.tile([128, S], BF16, tag="vt")
        v_sb = act.tile([128, NST, 128], BF16, tag="v")  # [s,d] chunks per k-tile
        attn_sb = [
            act.tile([128, S], BF16, tag=f"attn{h}", name=f"attn{h}")
            for h in range(NQ)
        ]
        wqkv_p = stack.enter_context(tc.tile_pool(name="wqkv", bufs=1))
        hstp = stack.enter_context(tc.tile_pool(name="hstp", bufs=2))
        rope = stack.enter_context(tc.tile_pool(name="rope", bufs=2))
        probs_p = stack.enter_context(tc.tile_pool(name="probs", bufs=3))
        den_p = stack.enter_context(tc.tile_pool(name="den", bufs=2))
        bcast_p = stack.enter_context(tc.tile_pool(name="bcast", bufs=2))
        ostage = stack.enter_context(tc.tile_pool(name="ostage", bufs=6))

        # ---- PSUM pools: 2 + 4 + 2 = 8 banks ----
        psA = stack.enter_context(tc.tile_pool(name="psA", bufs=2, space="PSUM"))
        psS = stack.enter_context(tc.tile_pool(name="psS", bufs=2, space="PSUM"))
        psO = stack.enter_context(tc.tile_pool(name="psO", bufs=2, space="PSUM"))

        # ================= prologue DMAs =================
        # Consumption order is chunk-major, so both rings stream c-groups
        # in lockstep:
        #   sync ring:   wqkv c-groups (then hst2/hst3/wo later)
        #   scalar ring: hst0 c-groups, cos/sin[qr0], hst1, cos/sin rest
        # Leading 2-chunk groups let the first matmuls start as soon as
        # the first ~650KB lands.
        hst_tiles: list = [None] * NQR
        hst_t0 = hstp.tile([128, KC, 512], BF16, tag="hst", name="hst0")
        hst_tiles[0] = hst_t0
        wqkv_sb = wqkv_p.tile([128, KC, 6, D], BF16)
        cos_sb = const.tile([128, S], BF16)
        sin_sb = const.tile([128, S], BF16)

        # 2-chunk groups: the k-stream in A(0) runs up to 10 chunks
        # ahead of the step index, so supply must track need at fine
        # grain (4-chunk groups put c~16-24 ~2us late).  The leading
        # 1-chunk groups + step-0's c0-first emission order let the
        # first matmul start ~1.5us earlier.  wqkv and hst0 groups
        # ALTERNATE across both rings: a single ring carrying all of
        # wqkv (6MB) runs exactly at its ~210GB/s cap with zero margin,
        # so any latency blip stalls the c-loop; interleaving gives
        # every chunk ~3us of slack.
        c_groups = [(0, 1), (1, 2)] + [(c, c + 2) for c in range(2, KC, 2)]
        for gi, (lo, hi) in enumerate(c_groups):
            wq_ring, hs_ring = (
                (nc.sync, nc.scalar) if gi % 2 == 0 else (nc.scalar, nc.sync)
            )
            wq_ring.dma_start(wqkv_sb[:, lo:hi], wqkv_d[:, lo:hi])
            hs_ring.dma_start(hst_t0[:, lo:hi, :], hst_d[0, :, lo:hi, :])
        nc.scalar.dma_start(cos_sb[:, 0:512], cos_d[:, 0:512])
        nc.scalar.dma_start(sin_sb[:, 0:512], sin_d[:, 0:512])
        nc.scalar.dma_start(cos_sb[:, 512:S], cos_d[:, 512:S])
        nc.scalar.dma_start(sin_sb[:, 512:S], sin_d[:, 512:S])
        # hst1 rides both rings AFTER the A(0) groups: its transfers
        # would otherwise steal aggregate DMA bandwidth from A(0)'s
        # supply-critical chunks; A(1) only needs it from ~56us.
        hst_t1 = hstp.tile([128, KC, 512], BF16, tag="hst", name="hst1")
        hst_tiles[1] = hst_t1
        for r in range(4):
            eng = nc.sync if r % 2 == 0 else nc.scalar
            eng.dma_start(
                hst_t1[:, r * 8 : (r + 1) * 8, :],
                hst_d[1, :, r * 8 : (r + 1) * 8, :],
            )

        def hst_chunks(dst, qr):
            for r in range(4):
                nc.sync.dma_start(
                    dst[:, r * 8 : (r + 1) * 8, :],
                    hst_d[qr, :, r * 8 : (r + 1) * 8, :],
                )

        # wo is allocated late, into hst2's hstp slot (dead once A(2) is
        # emitted) — SBUF is too tight to hold both for the whole kernel.
        wo_ref: dict = {}

        # weight chunk views: j=0 -> wk, j=1 -> wv, j=2+h -> wq head h
        def w_of(kind, h, c):
            if kind == "k":
                return wqkv_sb[:, c, 0, :]
            if kind == "v":
                return wqkv_sb[:, c, 1, :]
            return wqkv_sb[:, c, 2 + h, :]

        # ---- gpsimd-built constants ----
        identity = const.tile([128, 128], BF16)
        make_identity(nc, identity[:])
        ones_b = const.tile([128, 1], BF16)
        nc.gpsimd.memset(ones_b[:], 1.0)
        # triangular mask for the diagonal 128x128 subtile: rows are k,
        # cols are q; keep q >= k.
        tri = const.tile([128, 128], BF16)
        nc.gpsimd.memset(tri[:], 1.0)
        nc.gpsimd.affine_select(
            out=tri[:],
            in_=tri[:],
            pattern=[[1, 128]],
            compare_op=mybir.AluOpType.is_ge,
            fill=0.0,
            base=0,
            channel_multiplier=-1,
        )
        # pswap: permutation matrix swapping partition halves, so the RoPE
        # rotate-half is one PE matmul (no DVE-queue stall)
        pswap = const.tile([128, 128], BF16)
        ptmp = const.tile([128, 128], BF16)
        nc.gpsimd.memset(pswap[:], 1.0)
        nc.gpsimd.memset(ptmp[:], 1.0)
        nc.gpsimd.affine_select(
            out=pswap[:],
            in_=pswap[:],
            pattern=[[1, 128]],
            compare_op=mybir.AluOpType.is_equal,
            fill=0.0,
            base=64,
            channel_multiplier=-1,
        )
        nc.gpsimd.affine_select(
            out=ptmp[:],
            in_=ptmp[:],
            pattern=[[1, 128]],
            compare_op=mybir.AluOpType.is_equal,
            fill=0.0,
            base=-64,
            channel_multiplier=-1,
        )
        nc.gpsimd.tensor_add(pswap[:], pswap[:], ptmp[:])

        # ================= A-phase building blocks =================
        def rope_copy(ps):
            """First half of a rope evict: snapshot the PSUM accumulator
            to SBUF.  Split from rope_rest so the pswap matmul can be
            emitted a step later and never waits on the DVE."""
            raw = rope.tile([128, 512], BF16, tag="raw")
            nc.vector.tensor_copy(raw[:], ps[:])
            return raw

        def rope_rest(raw, ps, dst_tile, qr):
            """dst[0:64]  = x0*cos - x1*sin
            dst[64:128] = x1*cos + x0*sin   (x0=ps[0:64], x1=ps[64:128]).
            The rotate-half is a pswap permutation matmul back into the
            job's own (now dead) PSUM accumulator — no DMA, and no wait
            embedded in the in-order DVE queue."""
            sl = slice(qr * 512, (qr + 1) * 512)
            nc.tensor.matmul(
                ps[:], pswap[:], raw[:], start=True, stop=True,
                skip_group_check=True,
            )
            m1 = rope.tile([128, 512], BF16, tag="m1")
            nc.vector.tensor_mul(m1[:], raw[:], cos_sb[:, sl])
            m2 = rope.tile([128, 512], BF16, tag="m2")
            nc.vector.tensor_mul(m2[:], ps[:], sin_sb[:, sl])
            # sin table's top half is pre-negated host-side, so the
            # combine is one full-width add instead of sub + add
            nc.vector.tensor_add(dst_tile[:, sl], m1[:], m2[:])

        def rope_evict(ps, dst_tile, qr):
            rope_rest(rope_copy(ps), ps, dst_tile, qr)

        def emit_a0():
            """Chunk-major A(0): per step, ~one matmul per job against
            the combined wqkv chunk.  Job finish-steps are staggered two
            steps apart (k@21, q0@23, q1@25, v@27, q2@29, q3@31, via
            doubled chunks on early steps) so each ~2.6us DVE rope-evict
            chain hides under the next job's remaining matmul stream —
            at full-chunk-major the six serial chains stall the in-order
            PE queue ~7us at the A(0)->B(0) boundary.  Returns thunks
            (q3 rope tail, last transposes) for B(0)'s fill slots."""
            ps_k = psA.tile([128, 512], F32, tag="a", name="a0_k")
            ps_v = psA.tile([128, 512], F32, tag="a", name="a0_v")
            ps_q01 = psS.tile([128, 1024], F32, tag="s", name="a0_q01")
            ps_q23 = psS.tile([128, 1024], F32, tag="s", name="a0_q23")
            raws: dict = {}
            order = ["k", "q0", "q1", "v", "q2", "q3"]
            streams = {
                "k": (ps_k, slice(0, 512), "k", 0),
                "v": (ps_v, slice(0, 512), "v", 0),
                "q0": (ps_q01, slice(0, 512), "q", 0),
                "q1": (ps_q01, slice(512, 1024), "q", 1),
                "q2": (ps_q23, slice(0, 512), "q", 2),
                "q3": (ps_q23, slice(512, 1024), "q", 3),
            }
            fin = {s: 21 + 2 * j for j, s in enumerate(order)}
            dbl = {s: KC - 1 - fin[s] for s in order}  # doubled early steps

            def mm(sname, c):
                ps, cols, kind, h = streams[sname]
                nc.tensor.matmul(
                    ps[:, cols],
                    w_of(kind, h, c),
                    hst_t0[:, c, :],
                    start=(c == 0),
                    stop=(c == KC - 1),
                    skip_group_check=True,
                )

            def evict_begin(sname):
                ps, cols, kind, _ = streams[sname]
                if kind == "v":
                    nc.vector.tensor_copy(vt_sb[:, 0:512], ps[:, cols])
                else:
                    raws[sname] = rope_copy(ps[:, cols])

            def evict_end(sname):
                ps, cols, kind, _ = streams[sname]
                if kind == "v":
                    return
                dst = kt_sb if kind == "k" else qt_sb[streams[sname][3]]
                rope_rest(raws[sname], ps[:, cols], dst, 0)

            def transp(kt):
                pst = psA.tile([128, 128], BF16, tag="a", name=f"vt{kt}")
                nc.tensor.transpose(
                    pst[:], vt_sb[:, kt * 128 : (kt + 1) * 128], identity[:]
                )
                nc.vector.tensor_copy(v_sb[:, kt, :], pst[:])

            for step in range(KC):
                if step == 0:
                    # c0 across all streams first: the first matmuls only
                    # need the leading 1-chunk DMA group
                    for sname in order:
                        mm(sname, 0)
                    for sname in order:
                        if dbl[sname] > 0:
                            mm(sname, 1)
                else:
                    for sname in order:
                        d, f = dbl[sname], fin[sname]
                        if step < d:
                            mm(sname, 2 * step)
                            mm(sname, 2 * step + 1)
                        elif step <= f:
                            mm(sname, step + d)
                for sname in order:
                    if step == fin[sname]:
                        evict_begin(sname)
                    elif step == fin[sname] + 1:
                        evict_end(sname)
                if step >= 29:
                    transp(step - 29)
            # q3's evict_begin fired at step 31; its rope tail and the
            # last transpose go first into section 0's fill slots

            def rest_q3():
                evict_end("q3")

            def transp3():
                transp(3)

            return [("pe", rest_q3), ("pe", transp3)]

        def a_units(qr):
            """Yield thunks for A(qr), qr>=1: 6 projection jobs in
            c-chunks of 8 matmuls, evictions, and the v transposes for
            this qr.  Each job's evict is delayed until after the NEXT
            job's first chunk so the DVE's PSUM read (which frees the
            accumulator bank) has slack."""
            hst_t = hst_tiles[qr]
            jobs = [("k", 0), ("v", 0)] + [("q", h) for h in range(NQ)]
            state: dict = {}
            pending = []

            for kind, h in jobs:

                def alloc(kind=kind, h=h):
                    state[(kind, h)] = psA.tile(
                        [128, 512], F32, tag="a", name=f"a{qr}_{kind}{h}"
                    )

                for cg in range(4):

                    def chunk(kind=kind, h=h, cg=cg, alloc=alloc):
                        if cg == 0:
                            alloc()
                        ps = state[(kind, h)]
                        for c in range(cg * 8, cg * 8 + 8):
                            nc.tensor.matmul(
                                ps[:],
                                w_of(kind, h, c),
                                hst_t[:, c, :],
                                start=(c == 0),
                                stop=(c == KC - 1),
                                skip_group_check=True,
                            )

                    yield ("pe", chunk)
                    if cg == 0 and pending:
                        for u in pending:
                            yield u
                        pending = []

                def evict(kind=kind, h=h):
                    ps = state.pop((kind, h))
                    if kind == "q":
                        rope_evict(ps, qt_sb[h], qr)
                    elif kind == "k":
                        rope_evict(ps, kt_sb, qr)
                    else:
                        sl = slice(qr * 512, (qr + 1) * 512)
                        nc.vector.tensor_copy(vt_sb[:, sl], ps[:])

                if kind == "v":
                    # v's evict + transposes stay immediate: the transpose
                    # scratch tiles must chain through both psA slots
                    # before the next job's accumulator is allocated
                    yield ("dve", evict)
                    for kt in range(qr * 4, qr * 4 + 4):

                        def transp(kt=kt):
                            pst = psA.tile(
                                [128, 128], BF16, tag="a", name=f"vt{kt}"
                            )
                            nc.tensor.transpose(
                                pst[:],
                                vt_sb[:, kt * 128 : (kt + 1) * 128],
                                identity[:],
                            )
                            nc.vector.tensor_copy(v_sb[:, kt, :], pst[:])

                        yield ("pe", transp)
                else:
                    pending.append(("dve", evict))

            for u in pending:
                yield u

        # ================= o_proj (phase C) machinery =================
        # the very last (st, ho) tile is split into 128-col pieces so the
        # final evict+DMA chain after the final matmul is short
        def c_units():
            for qrC in range(NQR):
                for st in range(qrC * 4, qrC * 4 + 4):
                    for ho in range(NHO):
                        last = st == NST - 1 and ho == NHO - 1
                        pieces = (
                            [(0, 512)] if not last else [(0, 256), (256, 512)]
                        )
                        for c0, c1 in pieces:
                            yield ("alloc", qrC, st, ho, c0, c1)
                            for h in range(NQ):
                                yield ("mm", qrC, st, ho, c0, c1, h)
                            yield ("evict", qrC, st, ho, c0, c1)

        c_state = {"gen": c_units(), "pending": None, "tile": None, "nalloc": 0,
                   "nevict": 0}

        def c_alloc_tile(st, ho, w, wide):
            """Rotate accumulators over psA only (fill mode) or all three
            PSUM pools (drain mode)."""
            i = c_state["nalloc"]
            c_state["nalloc"] += 1
            if not wide:
                return psA.tile([128, w], F32, tag="a", name=f"c{st}_{ho}")
            # drain mode: rotate over all 8 banks (2 per pool tag, the
            # "s" tiles are 2 banks each) so bank reuse is ~3 units out
            which = i % 6
            if which in (0, 3):
                return psA.tile([128, w], F32, tag="a", name=f"c{st}_{ho}")
            if which in (1, 4):
                return psO.tile([128, w], F32, tag="o", name=f"c{st}_{ho}")
            return psS.tile([128, 1024], F32, tag="s", name=f"c{st}_{ho}")

        def emit_c(n_mms, qr_done, wide=False):
            emitted = 0
            while emitted < n_mms:
                unit = c_state["pending"] or next(c_state["gen"], None)
                c_state["pending"] = None
                if unit is None:
                    return False
                if unit[1] > qr_done:
                    c_state["pending"] = unit
                    return False
                if unit[0] == "alloc":
                    _, _, st, ho, c0, c1 = unit
                    c_state["tile"] = c_alloc_tile(st, ho, c1 - c0, wide)
                elif unit[0] == "mm":
                    _, _, st, ho, c0, c1, h = unit
                    nc.tensor.matmul(
                        c_state["tile"][:, 0 : c1 - c0],
                        attn_sb[h][:, st * 128 : (st + 1) * 128],
                        wo_ref["wo"][:, h, ho * 512 + c0 : ho * 512 + c1],
                        start=(h == 0),
                        stop=(h == NQ - 1),
                        skip_group_check=True,
                    )
                    emitted += 1
                else:
                    _, _, st, ho, c0, c1 = unit
                    i = c_state["nevict"]
                    c_state["nevict"] += 1
                    w = c1 - c0
                    stg = ostage.tile([128, w], BF16, tag="stg")
                    if not wide or i % 3 != 1:
                        # fill mode keeps ACT free — it paces the B(3) exps
                        nc.vector.tensor_copy(stg[:], c_state["tile"][:, 0:w])
                    else:
                        nc.scalar.copy(stg[:], c_state["tile"][:, 0:w])
                    # drain outputs rotate over sync/scalar/gpsimd (all
                    # inputs done by then; gpsimd's queue is empty, so its
                    # triggers fire the moment the stage copy lands);
                    # fills ride sync (scalar is still running B(3) exps).
                    # The last few tiles avoid gpsimd — its end-of-kernel
                    # queue DRAIN detects completion ~2us slower than the
                    # rings and would gate the postamble.
                    if not wide:
                        eng = nc.sync
                    elif i > 120:
                        eng = (nc.sync, nc.scalar)[i % 2]
                    else:
                        eng = (nc.sync, nc.scalar, nc.gpsimd)[i % 3]
                    eng.dma_start(
                        out_d[
                            st * 128 : (st + 1) * 128,
                            ho * 512 + c0 : ho * 512 + c1,
                        ],
                        stg[:],
                    )
            return True

        # ================= B-phase building blocks =================
        # (note: moving the denominator partition-reduce to gpsimd
        # partition_all_reduce was tried twice and regresses ~20us: its
        # ~3.5us latency lands in the in-order DVE queue — the DVE runs
        # AHEAD of real time on the folds, so even a head of emission
        # distance doesn't hide it, and every PE wait on a DVE count
        # slips.  The PE ones-matmul is in-stream and effectively free.)
        def b_units(qr):
            """Yield thunks for the attention inner loop of q-range qr.
            Each pair-visit thunk emits: scores prefetch for the next
            pair + exp, then (after the scheduler's filler) PV + folds."""
            n_kt = 4 * (qr + 1)
            n_pair = n_kt // 2
            qsl = slice(qr * 512, (qr + 1) * 512)

            def c0_of(kt):
                p_idx = kt - 4 * qr
                return 128 * p_idx if p_idx > 0 else 0

            for h in range(NQ):
                st8: dict = {}

                def mm_scores_pair(j, h=h, st8=st8):
                    ps_s = psS.tile(
                        [128, 1024], F32, tag="s", name=f"s{qr}_{h}_{j}"
                    )
                    for idx in range(2):
                        kt = 2 * j + idx
                        c0 = c0_of(kt)
                        nc.tensor.matmul(
                            ps_s[:, idx * 512 + c0 : (idx + 1) * 512],
                            kt_sb[:, kt * 128 : (kt + 1) * 128],
                            qt_sb[h][:, qr * 512 + c0 : (qr + 1) * 512],
                            start=True,
                            stop=True,
                            skip_group_check=True,
                        )
                    st8[j] = ps_s

                def act_exp(j, h=h, st8=st8):
                    ps_s = st8[j]
                    pt = probs_p.tile(
                        [128, 1024], BF16, tag="pt", name=f"pt{qr}_{h}_{j}"
                    )
                    if 2 * j + 1 < 4 * qr:
                        nc.scalar.activation(
                            pt[:],
                            ps_s[:],
                            mybir.ActivationFunctionType.Exp,
                            scale=SCALE,
                        )
                    else:
                        for idx in range(2):
                            kt = 2 * j + idx
                            c0 = c0_of(kt)
                            nc.scalar.activation(
                                pt[:, idx * 512 + c0 : (idx + 1) * 512],
                                ps_s[:, idx * 512 + c0 : (idx + 1) * 512],
                                mybir.ActivationFunctionType.Exp,
                                scale=SCALE,
                            )
                            nc.vector.tensor_mul(
                                pt[:, idx * 512 + c0 : idx * 512 + c0 + 128],
                                pt[:, idx * 512 + c0 : idx * 512 + c0 + 128],
                                tri[:],
                            )
                    st8[("pt", j)] = pt

                def pv_folds(j, h=h, st8=st8):
                    pt = st8.pop(("pt", j))
                    ps_o = st8["o"]
                    den_acc = st8["d"]
                    for idx in range(2):
                        kt = 2 * j + idx
                        c0 = c0_of(kt)
                        nc.tensor.matmul(
                            ps_o[:, c0:512],
                            v_sb[:, kt, :],
                            pt[:, idx * 512 + c0 : (idx + 1) * 512],
                            start=(kt == 0),
                            stop=(kt == n_kt - 1),
                            skip_group_check=True,
                        )
                    pts = probs_p.tile(
                        [128, 512], BF16, tag="pts", name=f"pts{qr}_{h}_{j}"
                    )
                    c0a, c0b = c0_of(2 * j), c0_of(2 * j + 1)
                    if c0b > c0a:
                        nc.vector.tensor_copy(pts[:, c0a:c0b], pt[:, c0a:c0b])
                    nc.vector.tensor_add(
                        pts[:, c0b:512],
                        pt[:, c0b:512],
                        pt[:, 512 + c0b : 1024],
                    )
                    if j == 0:
                        nc.vector.tensor_copy(den_acc[:], pts[:])
                    else:
                        nc.vector.tensor_add(
                            den_acc[:, c0a:512],
                            den_acc[:, c0a:512],
                            pts[:, c0a:512],
                        )

                def head_begin(h=h, st8=st8, msp=mm_scores_pair, ae=act_exp):
                    st8["o"] = psO.tile(
                        [128, 512], F32, tag="o", name=f"o{qr}_{h}"
                    )
                    # bf16 accumulator: the DVE folds hit 2x mode.  Costs
                    # ~0.5% on the denominator (sqrt(n_pair) bf16 rounds).
                    st8["d"] = den_p.tile(
                        [128, 512], BF16, tag="da", name=f"da{qr}_{h}"
                    )
                    msp(0)
                    ae(0)

                yield ("pe", head_begin)
                # filler right after the first scores pair: keeps the PE
                # fed while ACT runs exp(0) at head/section starts
                yield ("fill", None)
                for j in range(n_pair):

                    def prefetch(j=j, n_pair=n_pair, msp=mm_scores_pair,
                                 ae=act_exp):
                        if j + 1 < n_pair:
                            msp(j + 1)
                            ae(j + 1)

                    yield ("pe", prefetch)
                    yield ("fill", None)

                    def fin(j=j, pf=pv_folds):
                        pf(j)

                    yield ("pe", fin)

                def head_end(h=h, st8=st8, n_pair=n_pair):
                    ps_s = st8[n_pair - 1]
                    den_acc = st8["d"]
                    # partition-reduce at bf16 matmul speed (1 cycle/row
                    # vs 4 for fp32)
                    nc.tensor.matmul(
                        ps_s[0:1, 0:512],
                        ones_b[:],
                        den_acc[:],
                        start=True,
                        stop=True,
                        skip_group_check=True,
                    )
                    recip = den_p.tile([1, 512], F32, tag="recip", name=f"rc{qr}_{h}")
                    nc.vector.reciprocal_approx_fast(out=recip[:], in_=ps_s[0:1, 0:512])
                    bc = bcast_p.tile([128, 512], F32, tag="bc")
                    nc.gpsimd.partition_broadcast(bc[:], recip[:])
                    nc.vector.tensor_mul(
                        attn_sb[h][:, qsl], st8["o"][:], bc[:]
                    )

                yield ("pe", head_end)

        # ================= the pipeline =================
        # Solid A(0) first (nothing to overlap with yet); its v
        # transposes are deferred into section 0's fill slots.
        a0_transp = emit_a0()

        for qr in range(NQR):
            if qr + 2 < NQR:
                nxt = hstp.tile(
                    [128, KC, 512], BF16, tag="hst", name=f"hst{qr+2}"
                )
                hst_tiles[qr + 2] = nxt
                hst_chunks(nxt, qr + 2)

            au = list(a_units(qr + 1)) if qr + 1 < NQR else []
            if qr == 0:
                # interleave A(1)'s k chunks (dependency-free PE cover)
                # with A(0)'s rope tail + last transpose, which wait on
                # the still-draining DVE evict chains
                au = [
                    au[0], a0_transp[0], au[1], a0_transp[1], au[2],
                ] + au[3:]
            bu = list(b_units(qr))
            n_fill = sum(1 for k, _ in bu if k == "fill")
            ai = 0
            fills_done = 0
            for k, thunk in bu:
                if k == "fill":
                    fills_done += 1
                    if au:
                        # distribute A units evenly over the fill slots
                        target = (len(au) * fills_done) // n_fill
                        while ai < target:
                            au[ai][1]()
                            ai += 1
                    else:
                        emit_c(2, qr - 1, wide=False)
                else:
                    thunk()
            while ai < len(au):
                au[ai][1]()
                ai += 1

            if qr == 1:
                # A(2) fully emitted — hst2's slot is reclaimable for wo
                wo_sb = hstp.tile([128, NQ, HID], BF16, tag="hst", name="wo_sb")
                wo_ref["wo"] = wo_sb
                for h in range(NQ):
                    nc.sync.dma_start(wo_sb[:, h, :], wo_d[:, h, :])

        # ---- drain the remaining o_proj work ----
        while emit_c(4, NQR - 1, wide=True):
            pass

    nc.compile()
    return nc


def _get_nc():
    if "nc" not in _CACHE:
        _CACHE["nc"] = _build_nc()
    return _CACHE["nc"]


def _bf16(x):
    return np.ascontiguousarray(x.astype(ml_dtypes.bfloat16))


def _prep_in_maps(hidden_states, sin_table, cos_table, Wq, Wk, Wv, Wo):
    hs0 = np.asarray(hidden_states, np.float32).reshape(S, HID)
    # hst[qr, p, c, s] = hs0[qr*512 + s, c*128 + p]
    hst = _bf16(hs0.reshape(NQR, 512, KC, 128).transpose(0, 3, 2, 1))
    cosT = np.asarray(cos_table, np.float32).T  # [64, S]
    sinT = np.asarray(sin_table, np.float32).T
    cos2 = _bf16(np.concatenate([cosT, cosT], 0))  # [128, S]
    # top half negated: rope combine is then a single full-width add
    sin2 = _bf16(np.concatenate([-sinT, sinT], 0))
    Wq = np.asarray(Wq, np.float32)
    Wk = np.asarray(Wk, np.float32)
    Wv = np.asarray(Wv, np.float32)
    Wo = np.asarray(Wo, np.float32)

    in_maps = []
    for c in range(N_CORES):
        wq_c = Wq[:, c * 512 : (c + 1) * 512]  # 4 q heads
        wk_c = Wk[:, c * 128 : (c + 1) * 128]  # 1 kv head
        wv_c = Wv[:, c * 128 : (c + 1) * 128]
        wo_c = Wo[c * 512 : (c + 1) * 512, :]  # matching rows
        # combined chunk-major weights: wqkv[p, c, j, d] with j = 0:k,
        # 1:v, 2+h:q head h; element = W_j[c*128 + p, (head offset+) d]
        wqkv = np.empty((128, KC, 6, D), np.float32)
        wqkv[:, :, 0, :] = wk_c.reshape(KC, 128, D).swapaxes(0, 1)
        wqkv[:, :, 1, :] = wv_c.reshape(KC, 128, D).swapaxes(0, 1)
        wq_l = wq_c.reshape(KC, 128, NQ, D).transpose(2, 1, 0, 3)  # [h,p,c,d]
        for h in range(NQ):
            wqkv[:, :, 2 + h, :] = wq_l[h]
        in_maps.append(
            {
                "hst": hst,
                "wqkv": _bf16(wqkv),
                "wo": _bf16(wo_c.reshape(NQ, 128, HID).swapaxes(0, 1)),
                "cos2": cos2,
                "sin2": sin2,
            }
        )
    return in_maps


def run(trace=False, **inputs):
    nc = _get_nc()
    in_maps = _prep_in_maps(**inputs)
    res = run_bass_kernel_spmd(
        nc, in_maps, core_ids=list(range(N_CORES)), trace=trace
    )
    partials = np.stack(
        [np.asarray(res.results[c]["out"], np.float32) for c in range(N_CORES)]
    )
    out = partials.sum(axis=0, dtype=np.float32).reshape(1, S, HID)
    return out, res


def kernel(**inputs):
    out, _ = run(trace=False, **inputs)
    return out
